# revision 32
# baseline (speedup 1.0000x reference)
"""Trainium2 Bass kernel for the combined Tacotron-style loss.

Strategy (pure data parallel, 8 samples per core on 8 NeuronCores).

Every loss term is a big reduction, so the kernel is built around moving as
few HBM bytes as possible and reducing them on the widest engines:

  - mel L1 terms: mo/mt/mp stream in fp8 (statistically safe for a 10M-element
    mean at 2e-2 tol).  The PE computes (mo-mt) and (mt-mp) with a +I/-I
    DoubleRow fp8 matmul into PSUM f32; ACT (Abs + accum) and DVE
    (tensor_reduce abs-add) split the row-sum work.
  - attention / guided-attention box terms: alignment rows are normalized
    (sum_j A[i,j] == 1), so sums over wide row prefixes are computed as
    1 - (narrow tail sum).  The host packs exactly the needed tail/window
    elements into a [128, D*512] fp8 "canvas"; a ones-stationary DoubleRow
    matmul chain column-sums it into one PSUM bank.  Column index mod 512
    identifies the group (box-tail / att-direct / att-tail) on the host.
  - gaussian term: sigma=0.4 makes exp(-(i-j*out/in)^2/(2s^2)) a <=4-column
    band; host gathers band values + weights, one DVE mult+accum reduces it.
  - gate BCE: f32, ACT Abs/Exp/Ln + Relu + DVE x*z, all with fused accum.

Host combines all partial sums in float64.
"""

import ml_dtypes
import numpy as np

import concourse.bacc as bacc
import concourse.mybir as mybir
from concourse import bass
from concourse.bass_utils import run_bass_kernel_spmd
from concourse.tile import TileContext

F32 = mybir.dt.float32
BF16 = mybir.dt.bfloat16
F8 = mybir.dt.float8e4
ALU = mybir.AluOpType
ACTF = mybir.ActivationFunctionType
DR = mybir.MatmulPerfMode.DoubleRow

F8NP = ml_dtypes.float8_e4m3
BFNP = ml_dtypes.bfloat16

# Problem shapes (hardcoded per contract).
B, MEL, TOUT, TIN = 64, 80, 2000, 400
NCORES = 8
BPC = B // NCORES                  # samples per core
MROWS = BPC * MEL                  # 640 mel rows per core
NMT = MROWS // 128                 # 5 mel row-tiles
GCOLS = BPC * TOUT // 128          # 125 gate cols ([128, 125] layout)
BW = 4                             # gaussian band width
SIGMA = 0.4
ESCALE = -1.0 / (2.0 * SIGMA * SIGMA)
MEL_W, GATE_W, ATT_W, GA_W = 1.0, 1.0, 0.1, 0.1
ASCALE = 16384.0                   # 2**14: puts fp8 alignment values in normal range

IMID = TIN // 2                    # 200: att rows i<=IMID summed directly,
#                                    i>IMID via 1 - tail
N_DIR = BPC * (IMID * (IMID + 1) // 2)          # direct window elements/core
N_ATT_TAIL = BPC * ((IMID - 1) * IMID // 2)     # att tail elements/core
ATT_CONST = (TOUT - TIN) + (TIN - 1 - IMID)     # exact-1.0 rows per sample

# att-direct mask: rows i=0..IMID, cols j<i  (j <= IMID-1)
_DIR_MASK = np.arange(IMID)[None, :] < np.arange(IMID + 1)[:, None]
# att-tail mask: rows i=IMID+1..TIN-1, cols j>=i
_TAIL_MASK = (np.arange(TIN)[None, :]
              >= (IMID + 1 + np.arange(TIN - 1 - IMID))[:, None])

# mel chunk-read engine assignment: alternate ACT / DVE (GPSIMD cannot
# read PSUM, so it instead takes all the small SBUF-side reductions)
N_MEL_SLOTS = NMT * 4              # 5 tiles x 2 halves x 2 pairs
READER = ['A', 'D'] * 10
MELH = TOUT // 2                   # 1000 data cols per mel half-tile
MELS = 1008                        # padded plane stride (DoubleRow: %16 == 0)
MELW = 3 * MELS                    # half-tile width (mo | mt | mp planes)
SA_COLS = 16                       # ACT stats: 0..9 mel, 10 softplus
SD_COLS = 16                       # DVE stats: 0..9 mel
SP_COLS = 8                        # Pool stats: 0 x*z, 1 relu, 2 band

# Canvas layout: set lazily from the actual inputs (sizes depend on
# input/output lengths).  (n_chunks D, (a,b) col ranges per group, band cols)
_LAYOUT = None


def _canvas_layout(max_box, nb_cols):
    """Pick D (512-col canvas chunks) + column ranges for the 3 groups."""
    sizes = [max_box, N_DIR, N_ATT_TAIL]
    total = sum(sizes)
    d = max(2, -(-total // (128 * 512)))
    while True:
        cols = [-(-s // (128 * d)) for s in sizes]
        if sum(cols) <= 512:
            break
        d += 1
    ranges = []
    a = 0
    for c in cols:
        ranges.append((a, a + c))
        a += c
    nb = -(-nb_cols // 64) * 64
    return (d, tuple(ranges), nb)


def _build_program(d_chunks, nb, n_reps=1):
    nc = bacc.Bacc(
        "TRN2",
        target_bir_lowering=False,
        debug=False,
        enable_asserts=False,
        num_devices=NCORES,
    )

    # one packed byte tensor for all the small inputs:
    # [id 256B | gate f32 1000B | band bf16 2*nb | bw bf16 2*nb]
    auxw = 256 + 8 * GCOLS + 4 * nb
    d_aux = nc.dram_tensor("aux", (128, auxw), mybir.dt.uint8,
                           kind="ExternalInput").ap()
    d_mel = nc.dram_tensor("mel", (128, NMT * 2 * MELW), F8,
                           kind="ExternalInput").ap()
    d_cv = nc.dram_tensor("cv", (128, d_chunks * 512), F8,
                          kind="ExternalInput").ap()

    o_sa = nc.dram_tensor("sa", (128, SA_COLS), F32, kind="ExternalOutput").ap()
    o_sd = nc.dram_tensor("sd", (128, SD_COLS), F32, kind="ExternalOutput").ap()
    o_sp = nc.dram_tensor("sp", (128, SP_COLS), F32, kind="ExternalOutput").ap()
    o_cs = nc.dram_tensor("cs", (1, 512), F32, kind="ExternalOutput").ap()

    with TileContext(nc) as tc:
        with (
            tc.tile_pool(name="small", bufs=1) as sp,
            tc.tile_pool(name="cvp", bufs=3) as cvp,
            tc.tile_pool(name="melp", bufs=4) as melp,
            tc.tile_pool(name="scrp", bufs=2) as scrp,
            tc.tile_pool(name="pscrp", bufs=2) as pscrp,
            tc.tile_pool(name="pscs", bufs=1, space="PSUM") as pscs,
            tc.tile_pool(name="psmel", bufs=3, space="PSUM") as psmel,
        ):
            aux_sb = sp.tile([128, 256 + 8 * GCOLS + 4 * nb], mybir.dt.uint8)
            nc.sync.dma_start(out=aux_sb[:], in_=d_aux)
            id_sb = aux_sb[:, 0:256].bitcast(F8)
            gate_sb = aux_sb[:, 256:256 + 8 * GCOLS].bitcast(F32)
            b0 = 256 + 8 * GCOLS
            band_sb = aux_sb[:, b0:b0 + 2 * nb].bitcast(BF16)
            bw_sb = aux_sb[:, b0 + 2 * nb:b0 + 4 * nb].bitcast(BF16)

            # ones stationary for canvas colsums: DoubleRow requires the
            # k-pair dim stride to be a multiple of 16
            ones2 = sp.tile([128, 32], F8)
            nc.gpsimd.memset(ones2[:], 1.0)
            sa = sp.tile([128, SA_COLS], F32)
            nc.vector.memset(sa[:], 0.0)
            sd = sp.tile([128, SD_COLS], F32)
            nc.vector.memset(sd[:], 0.0)
            spst = sp.tile([128, SP_COLS], F32)
            nc.gpsimd.memset(spst[:], 0.0)

            cs_ps = pscs.tile([1, 512], F32)

            for _rep in range(n_reps):
                _emit_body(nc, sp, cvp, melp, scrp, pscrp, psmel,
                           id_sb, gate_sb, band_sb, bw_sb, ones2,
                           sa, sd, spst, cs_ps, d_cv, d_mel, d_chunks)

            # spread the output DMAs across queues so their issue/sem
            # latencies overlap instead of stacking on one SEQ
            cs_sb = sp.tile([1, 512], F32)
            nc.vector.tensor_copy(out=cs_sb[:], in_=cs_ps[:])
            nc.scalar.dma_start(out=o_cs, in_=cs_sb[:])
            nc.scalar.dma_start(out=o_sa, in_=sa[:])
            nc.sync.dma_start(out=o_sd, in_=sd[:])
            nc.gpsimd.dma_start(out=o_sp, in_=spst[:])

    nc.compile()
    return nc


def _emit_body(nc, sp, cvp, melp, scrp, pscrp, psmel,
               id_sb, gate_sb, band_sb, bw_sb, ones2,
               sa, sd, spst, cs_ps, d_cv, d_mel, d_chunks):
    nb = band_sb.shape[1]

    # --- gate BCE: |x| and exp(-|x|) early on ACT (same table set as mel's
    # Abs, so the only extra table load is Ln's, deferred to the tail);
    # x*z and relu sums on DVE ---
    go = gate_sb[:, 0:GCOLS]
    gt = gate_sb[:, GCOLS:2 * GCOLS]
    g1 = sp.tile([128, GCOLS], F32, tag="g1")
    nc.scalar.activation(out=g1[:], in_=go, func=ACTF.Abs)
    g2 = sp.tile([128, GCOLS], F32, tag="g2")
    nc.scalar.activation(out=g2[:], in_=g1[:], func=ACTF.Exp, scale=-1.0)
    g5 = sp.tile([128, GCOLS], F32, tag="g5")
    nc.vector.scalar_tensor_tensor(
        out=g5[:], in0=go, scalar=0.0, in1=gt,
        op0=ALU.add, op1=ALU.mult, accum_out=spst[:, 0:1])
    g6 = sp.tile([128, GCOLS], F32, tag="g6")
    nc.vector.scalar_tensor_tensor(
        out=g6[:], in0=go, scalar=0.0, in1=go,
        op0=ALU.is_gt, op1=ALU.mult, accum_out=spst[:, 1:2])
    bscr = sp.tile([128, nb], BF16, tag="bscr")
    nc.vector.scalar_tensor_tensor(
        out=bscr[:], in0=band_sb, scalar=1.0, in1=bw_sb,
        op0=ALU.mult, op1=ALU.mult, accum_out=spst[:, 2:3])

    ones_v = ones2[:].rearrange("p (two s) -> p two s", two=2)[:, :, 0:1]
    id2 = id_sb.rearrange("p (two m) -> p two m", two=2)

    # --- mel L1 (PE diffs -> ACT/DVE abs+row-sum).  All mel DMAs go first:
    # the psum readers are the long pole, so their stream starts ASAP.
    # The canvas only feeds the (cheap) PE colsum chain and rides last. ---
    ncols = {'A': 0, 'D': 0}
    for kh in range(NMT * 2):
        mt = melp.tile([128, MELW], F8, tag="mel")
        nc.sync.dma_start(out=mt[:], in_=d_mel[:, kh * MELW:(kh + 1) * MELW])
        for p in range(2):
            # pair 0: planes (mo, mt) -> mo - mt; pair 1: (mt, mp) -> mt - mp
            pv = mt[:, p * MELS:p * MELS + 2 * MELS].rearrange(
                "p (two j) -> p two j", two=2)
            ps = psmel.tile([128, 1024], F32, tag="mps")
            nc.tensor.matmul(ps[:, 0:512], id2, pv[:, :, 0:512],
                             start=True, stop=True, perf_mode=DR,
                             skip_group_check=True)
            nc.tensor.matmul(ps[:, 512:MELH], id2, pv[:, :, 512:MELH],
                             start=True, stop=True, perf_mode=DR,
                             skip_group_check=True)
            eng = READER[kh * 2 + p]
            col = ncols[eng]
            ncols[eng] += 1
            if eng == 'A':
                scr = scrp.tile([128, MELH], BF16, tag="scr")
                nc.scalar.activation(out=scr[:], in_=ps[:, 0:MELH],
                                     func=ACTF.Abs,
                                     accum_out=sa[:, col:col + 1])
            else:
                nc.vector.tensor_reduce(
                    out=sd[:, col:col + 1], in_=ps[:, 0:MELH],
                    axis=mybir.AxisListType.X, op=ALU.add,
                    apply_absolute_value=True)

    # --- canvas column sums (ones-stationary DoubleRow chain) ---
    total_cv = d_chunks * 512
    n_groups = (d_chunks + 1) // 2
    ones1 = ones2[:, 0:1]
    cv_pair = 0
    off = 0
    while off < total_cv:
        w = min(2048, total_cv - off)
        cvt = cvp.tile([128, 2048], F8, tag="cv")
        nc.sync.dma_start(out=cvt[:, 0:w], in_=d_cv[:, off:off + w])
        h = 0
        while h * 1024 < w:
            first = cv_pair == 0
            last = cv_pair == n_groups - 1
            if w - h * 1024 >= 1024:
                nc.tensor.matmul(
                    cs_ps[:], ones_v,
                    cvt[:, h * 1024:(h + 1) * 1024].rearrange(
                        "p (two j) -> p two j", two=2),
                    start=first, stop=last,
                    perf_mode=DR, skip_group_check=True)
            else:
                # odd trailing 512-col chunk: plain fp8 matmul
                nc.tensor.matmul(
                    cs_ps[:], ones1, cvt[:, h * 1024:h * 1024 + 512],
                    start=first, stop=last, skip_group_check=True)
            cv_pair += 1
            h += 1
        off += w

    # --- gate BCE tail: ln(1 + exp(-|x|)) accum (needs the natural_log
    # activation table; the load lands after the mel Abs stream ends) ---
    g3 = sp.tile([128, GCOLS], F32, tag="g3")
    nc.scalar.activation(out=g3[:], in_=g2[:], func=ACTF.Ln, bias=1.0,
                         accum_out=sa[:, 10:11])


_PROGRAMS = {}


def _get_program(d_chunks=None, nb=None, n_reps=1):
    if d_chunks is None or nb is None:
        assert _LAYOUT is not None, "call kernel() first"
        d_chunks, _, nb = _LAYOUT
    key = (d_chunks, nb, n_reps)
    if key not in _PROGRAMS:
        _PROGRAMS[key] = _build_program(d_chunks, nb, n_reps)
    return _PROGRAMS[key]


def _build_program_reps(n_reps):
    assert _LAYOUT is not None, "call kernel() (or _prep_core) first"
    d, _, nb = _LAYOUT
    return _get_program(d, nb, n_reps)


def _core_box_count(in_len, out_len):
    return int(np.sum(out_len.astype(np.int64) * (TIN - in_len.astype(np.int64))))


def _core_band_cols(out_len):
    return -(-int(np.sum(out_len.astype(np.int64))) * BW // 128)


def _prep_core(al, melo, melp_, melt, go, gt, in_len, out_len):
    """Build one core's input map. al: [BPC, TOUT, TIN] etc. (numpy f32)."""
    global _LAYOUT
    in_len = np.asarray(in_len, dtype=np.int64)
    out_len = np.asarray(out_len, dtype=np.int64)
    if _LAYOUT is None:
        # standalone use: size from this core with margin
        _LAYOUT = _canvas_layout(int(_core_box_count(in_len, out_len) * 1.25),
                                 _core_band_cols(out_len) + 64)
    d, ranges, nb = _LAYOUT

    # mel: per (row-tile k, half h): [mo | mt | mp] planes of MELH cols
    # padded to MELS so the DoubleRow plane stride is a multiple of 16
    m3 = np.stack([melo.reshape(MROWS, TOUT),
                   melt.reshape(MROWS, TOUT),
                   melp_.reshape(MROWS, TOUT)], axis=1)     # [640, 3, 2000]
    m4 = np.zeros((NMT, 128, 2, 3, MELS), np.float32)
    m5 = m3.reshape(NMT, 128, 3, 2, MELH)                   # [k, p, t, h, j]
    m4[:, :, :, :, 0:MELH] = m5.transpose(0, 1, 3, 2, 4)
    mel8 = np.ascontiguousarray(
        m4.transpose(1, 0, 2, 3, 4).reshape(128, NMT * 2 * MELW)).astype(F8NP)

    # canvas groups
    box_vals = [al[s, :out_len[s], in_len[s]:] for s in range(BPC)]
    box = (np.concatenate([v.ravel() for v in box_vals])
           if box_vals else np.zeros(0, np.float32))
    dirv = np.concatenate([al[s, :IMID + 1, :IMID][_DIR_MASK]
                           for s in range(BPC)])
    tailv = np.concatenate([al[s, IMID + 1:TIN, :][_TAIL_MASK]
                            for s in range(BPC)])

    cv = np.zeros((d, 512, 128), np.float32)
    for vals, (a, b) in zip((box, dirv, tailv), ranges):
        cap = d * (b - a) * 128
        assert len(vals) <= cap, f"canvas overflow: {len(vals)} > {cap}"
        pad = np.zeros(cap, np.float32)
        pad[:len(vals)] = vals * ASCALE
        cv[:, a:b, :] = pad.reshape(d, b - a, 128)
    cv8 = np.ascontiguousarray(cv.transpose(2, 0, 1).reshape(128, d * 512)
                               ).astype(F8NP)

    # gaussian band: 4 columns around j* = i*in/out for valid rows
    bands = []
    bws = []
    for s in range(BPC):
        ol, il = int(out_len[s]), int(in_len[s])
        iv = np.arange(ol, dtype=np.float64)
        jstar = iv * il / ol
        s0 = np.clip(np.floor(jstar).astype(np.int64) - 1, 0, TIN - BW)
        jb = s0[:, None] + np.arange(BW)[None, :]            # [ol, BW]
        bands.append(al[s, iv.astype(np.int64)[:, None], jb].ravel())
        dlt = iv[:, None] - jb * (float(ol) / il)
        w = np.exp(ESCALE * dlt * dlt)
        w[jb >= il] = 0.0
        bws.append(w.ravel())
    bflat = np.concatenate(bands)
    wflat = np.concatenate(bws)
    bpad = np.zeros(128 * nb, np.float32)
    bpad[:len(bflat)] = bflat
    wpad = np.zeros(128 * nb, np.float32)
    wpad[:len(wflat)] = wflat

    # identity stationary: [p, 0*128+m]=+1[p==m], [p, 128+m]=-1[p==m]
    idw = np.zeros((128, 256), np.float32)
    idw[np.arange(128), np.arange(128)] = 1.0
    idw[np.arange(128), 128 + np.arange(128)] = -1.0

    gate = np.ascontiguousarray(
        np.concatenate([go.reshape(128, GCOLS), gt.reshape(128, GCOLS)],
                       axis=1), dtype=np.float32)
    u8 = np.uint8
    aux = np.concatenate([
        np.ascontiguousarray(idw.astype(F8NP)).view(u8),
        gate.view(u8),
        np.ascontiguousarray(bpad.reshape(128, nb).astype(BFNP)).view(u8),
        np.ascontiguousarray(wpad.reshape(128, nb).astype(BFNP)).view(u8),
    ], axis=1)

    return {"aux": np.ascontiguousarray(aux), "mel": mel8, "cv": cv8}


def kernel(mel_out, mel_out_postnet, gate_out, alignments,
           mel_target, gate_target, input_lengths, output_lengths,
           _results_hook=None):
    global _LAYOUT
    mel_out = np.asarray(mel_out, dtype=np.float32)
    mel_out_postnet = np.asarray(mel_out_postnet, dtype=np.float32)
    gate_out = np.asarray(gate_out, dtype=np.float32)
    alignments = np.asarray(alignments, dtype=np.float32)
    mel_target = np.asarray(mel_target, dtype=np.float32)
    gate_target = np.asarray(gate_target, dtype=np.float32)
    in_len = np.asarray(input_lengths).astype(np.int64)
    out_len = np.asarray(output_lengths).astype(np.int64)

    # global layout from all cores (one SPMD program)
    max_box = 0
    max_nb = 0
    for c in range(NCORES):
        sl = slice(BPC * c, BPC * (c + 1))
        max_box = max(max_box, _core_box_count(in_len[sl], out_len[sl]))
        max_nb = max(max_nb, _core_band_cols(out_len[sl]))
    lay = _canvas_layout(max_box, max_nb)
    if _LAYOUT is None or _LAYOUT[0] < lay[0] or _LAYOUT[2] < lay[2]:
        _LAYOUT = lay
    d, ranges, nb = _LAYOUT

    in_maps = []
    for c in range(NCORES):
        sl = slice(BPC * c, BPC * (c + 1))
        in_maps.append(_prep_core(
            alignments[sl], mel_out[sl], mel_out_postnet[sl], mel_target[sl],
            gate_out[sl], gate_target[sl], in_len[sl], out_len[sl]))

    nc = _get_program(d, nb)
    res = run_bass_kernel_spmd(nc, in_maps, core_ids=list(range(NCORES)))
    if _results_hook is not None:
        _results_hook(res)

    mel_sum = gsp = grelu = gxz = gauss = 0.0
    att = box = 0.0
    (ba, bb), (da, db), (ta, tb) = ranges
    for c in range(NCORES):
        out = res.results[c]
        sa = out["sa"].astype(np.float64)
        sd = out["sd"].astype(np.float64)
        spst = out["sp"].astype(np.float64)
        cs = out["cs"].astype(np.float64)[0]

        mel_sum += sa[:, 0:10].sum() + sd[:, 0:10].sum()
        gsp += sa[:, 10].sum()
        gxz += spst[:, 0].sum()
        grelu += spst[:, 1].sum()
        gauss += spst[:, 2].sum()

        box_tail = cs[ba:bb].sum() / ASCALE
        att_dir = cs[da:db].sum() / ASCALE
        att_tail = cs[ta:tb].sum() / ASCALE

        sl = slice(BPC * c, BPC * (c + 1))
        att += BPC * ATT_CONST + att_dir - att_tail
        box += float(out_len[sl].sum()) - box_tail

    n_mel = B * MEL * TOUT
    n_gate = B * TOUT
    mel_loss = mel_sum / n_mel
    gate_loss = (grelu - gxz + gsp) / n_gate
    att_loss = att / B
    ga_loss = (box - gauss) / B
    total = (MEL_W * mel_loss + GATE_W * gate_loss
             + ATT_W * att_loss + GA_W * ga_loss)
    f = np.float32
    return (f(total), f(mel_loss), f(gate_loss), f(att_loss), f(ga_loss))


# revision 33
# speedup vs baseline: 1.1286x; 1.1286x over previous
"""Trainium2 Bass kernel for the combined Tacotron-style loss.

Strategy (pure data parallel, 8 samples per core on 8 NeuronCores).

Every loss term is a big reduction, so the kernel is built around moving as
few HBM bytes as possible and reducing them on the widest engines:

  - mel L1 terms: mo/mt/mp stream in fp8 (statistically safe for a 10M-element
    mean at 2e-2 tol).  The PE computes (mo-mt) and (mt-mp) with a +I/-I
    DoubleRow fp8 matmul into PSUM f32; ACT (Abs + accum) and DVE
    (tensor_reduce abs-add) split the row-sum work.
  - attention / guided-attention box terms: alignment rows are normalized
    (sum_j A[i,j] == 1), so sums over wide row prefixes are computed as
    1 - (narrow tail sum).  The host packs exactly the needed tail/window
    elements into a [128, D*512] fp8 "canvas"; a ones-stationary DoubleRow
    matmul chain column-sums it into one PSUM bank.  Column index mod 512
    identifies the group (box-tail / att-direct / att-tail) on the host.
  - gaussian term: sigma=0.4 makes exp(-(i-j*out/in)^2/(2s^2)) a <=4-column
    band; host gathers band values + weights, one DVE mult+accum reduces it.
  - gate BCE: f32, ACT Abs/Exp/Ln + Relu + DVE x*z, all with fused accum.

Host combines all partial sums in float64.
"""

import ml_dtypes
import numpy as np

import concourse.bacc as bacc
import concourse.mybir as mybir
from concourse import bass
from concourse.bass_utils import run_bass_kernel_spmd
from concourse.tile import TileContext

F32 = mybir.dt.float32
BF16 = mybir.dt.bfloat16
F8 = mybir.dt.float8e4
ALU = mybir.AluOpType
ACTF = mybir.ActivationFunctionType
DR = mybir.MatmulPerfMode.DoubleRow

F8NP = ml_dtypes.float8_e4m3
BFNP = ml_dtypes.bfloat16

# Problem shapes (hardcoded per contract).
B, MEL, TOUT, TIN = 64, 80, 2000, 400
NCORES = 8
BPC = B // NCORES                  # samples per core
MROWS = BPC * MEL                  # 640 mel rows per core
NMT = MROWS // 128                 # 5 mel row-tiles
GCOLS = BPC * TOUT // 128          # 125 gate cols ([128, 125] layout)
BW = 4                             # gaussian band width
SIGMA = 0.4
ESCALE = -1.0 / (2.0 * SIGMA * SIGMA)
MEL_W, GATE_W, ATT_W, GA_W = 1.0, 1.0, 0.1, 0.1
ASCALE = 16384.0                   # 2**14: puts fp8 alignment values in normal range

IMID = TIN // 2                    # 200: att rows i<=IMID summed directly,
#                                    i>IMID via 1 - tail
N_DIR = BPC * (IMID * (IMID + 1) // 2)          # direct window elements/core
N_ATT_TAIL = BPC * ((IMID - 1) * IMID // 2)     # att tail elements/core
ATT_CONST = (TOUT - TIN) + (TIN - 1 - IMID)     # exact-1.0 rows per sample

# att-direct mask: rows i=0..IMID, cols j<i  (j <= IMID-1)
_DIR_MASK = np.arange(IMID)[None, :] < np.arange(IMID + 1)[:, None]
# att-tail mask: rows i=IMID+1..TIN-1, cols j>=i
_TAIL_MASK = (np.arange(TIN)[None, :]
              >= (IMID + 1 + np.arange(TIN - 1 - IMID))[:, None])

# mel chunk-read engine assignment: alternate ACT / DVE (GPSIMD cannot
# read PSUM, so it instead takes all the small SBUF-side reductions)
N_MEL_SLOTS = NMT * 4              # 5 tiles x 2 halves x 2 pairs
READER = ['A', 'D'] * 10
MELH = TOUT // 2                   # 1000 data cols per mel half-tile
MELS = 1008                        # padded plane stride (DoubleRow: %16 == 0)
MELW = 3 * MELS                    # half-tile width (mo | mt | mp planes)
SA_COLS = 16                       # ACT stats: 0..9 mel, 10 softplus
SD_COLS = 16                       # DVE stats: 0..9 mel
SP_COLS = 8                        # Pool stats: 0 x*z, 1 relu, 2 band

# Canvas layout: set lazily from the actual inputs (sizes depend on
# input/output lengths).  (n_chunks D, (a,b) col ranges per group, band cols)
_LAYOUT = None


def _canvas_layout(max_box, nb_cols):
    """Pick D (512-col canvas chunks) + column ranges for the 3 groups."""
    sizes = [max_box, N_DIR, N_ATT_TAIL]
    total = sum(sizes)
    d = max(2, -(-total // (128 * 512)))
    while True:
        cols = [-(-s // (128 * d)) for s in sizes]
        if sum(cols) <= 512:
            break
        d += 1
    ranges = []
    a = 0
    for c in cols:
        ranges.append((a, a + c))
        a += c
    nb = -(-nb_cols // 64) * 64
    return (d, tuple(ranges), nb)


def _build_program(d_chunks, nb, n_reps=1):
    nc = bacc.Bacc(
        "TRN2",
        target_bir_lowering=False,
        debug=False,
        enable_asserts=False,
        num_devices=NCORES,
    )

    # one packed byte tensor for all the small inputs:
    # [id 256B | gate f32 1000B | band bf16 2*nb | bw bf16 2*nb]
    auxw = 256 + 8 * GCOLS + 4 * nb
    d_aux = nc.dram_tensor("aux", (128, auxw), mybir.dt.uint8,
                           kind="ExternalInput").ap()
    d_mel = nc.dram_tensor("mel", (128, NMT * 2 * MELW), F8,
                           kind="ExternalInput").ap()
    d_cv = nc.dram_tensor("cv", (128, d_chunks * 512), F8,
                          kind="ExternalInput").ap()

    o_sa = nc.dram_tensor("sa", (128, SA_COLS), F32, kind="ExternalOutput").ap()
    o_sd = nc.dram_tensor("sd", (128, SD_COLS), F32, kind="ExternalOutput").ap()
    o_sp = nc.dram_tensor("sp", (128, SP_COLS), F32, kind="ExternalOutput").ap()
    o_cs = nc.dram_tensor("cs", (1, 512), F32, kind="ExternalOutput").ap()

    n_cv_dmas = (d_chunks * 512 + 2047) // 2048
    with TileContext(nc) as tc:
        with (
            tc.tile_pool(name="small", bufs=1) as sp,
            # canvas rides after the mel stream and its matmuls queue behind
            # the reader-paced mel matmuls on the in-order PE: keep every
            # chunk resident so the canvas DMAs never stall on buf rotation
            tc.tile_pool(name="cvp", bufs=n_cv_dmas) as cvp,
            tc.tile_pool(name="melp", bufs=6) as melp,
            tc.tile_pool(name="scrp", bufs=2) as scrp,
            tc.tile_pool(name="pscrp", bufs=2) as pscrp,
            tc.tile_pool(name="pscs", bufs=1, space="PSUM") as pscs,
            tc.tile_pool(name="psmel", bufs=3, space="PSUM") as psmel,
        ):
            aux_sb = sp.tile([128, 256 + 8 * GCOLS + 4 * nb], mybir.dt.uint8)
            nc.sync.dma_start(out=aux_sb[:], in_=d_aux)
            id_sb = aux_sb[:, 0:256].bitcast(F8)
            gate_sb = aux_sb[:, 256:256 + 8 * GCOLS].bitcast(F32)
            b0 = 256 + 8 * GCOLS
            band_sb = aux_sb[:, b0:b0 + 2 * nb].bitcast(BF16)
            bw_sb = aux_sb[:, b0 + 2 * nb:b0 + 4 * nb].bitcast(BF16)

            # ones stationary for canvas colsums: DoubleRow requires the
            # k-pair dim stride to be a multiple of 16
            ones2 = sp.tile([128, 32], F8)
            nc.gpsimd.memset(ones2[:], 1.0)
            sa = sp.tile([128, SA_COLS], F32)
            nc.vector.memset(sa[:], 0.0)
            sd = sp.tile([128, SD_COLS], F32)
            nc.vector.memset(sd[:], 0.0)
            spst = sp.tile([128, SP_COLS], F32)
            nc.gpsimd.memset(spst[:], 0.0)

            cs_ps = pscs.tile([1, 512], F32)

            for _rep in range(n_reps):
                _emit_body(nc, sp, cvp, melp, scrp, pscrp, psmel,
                           id_sb, gate_sb, band_sb, bw_sb, ones2,
                           sa, sd, spst, cs_ps, d_cv, d_mel, d_chunks)

            # spread the output DMAs across queues so their issue/sem
            # latencies overlap instead of stacking on one SEQ
            cs_sb = sp.tile([1, 512], F32)
            nc.vector.tensor_copy(out=cs_sb[:], in_=cs_ps[:])
            nc.scalar.dma_start(out=o_cs, in_=cs_sb[:])
            nc.scalar.dma_start(out=o_sa, in_=sa[:])
            nc.sync.dma_start(out=o_sd, in_=sd[:])
            nc.gpsimd.dma_start(out=o_sp, in_=spst[:])

    nc.compile()
    return nc


def _emit_body(nc, sp, cvp, melp, scrp, pscrp, psmel,
               id_sb, gate_sb, band_sb, bw_sb, ones2,
               sa, sd, spst, cs_ps, d_cv, d_mel, d_chunks):
    nb = band_sb.shape[1]

    # --- gate BCE: |x| and exp(-|x|) early on ACT (same table set as mel's
    # Abs, so the only extra table load is Ln's, deferred to the tail);
    # x*z and relu sums on DVE ---
    go = gate_sb[:, 0:GCOLS]
    gt = gate_sb[:, GCOLS:2 * GCOLS]
    g1 = sp.tile([128, GCOLS], F32, tag="g1")
    nc.scalar.activation(out=g1[:], in_=go, func=ACTF.Abs)
    g2 = sp.tile([128, GCOLS], F32, tag="g2")
    nc.scalar.activation(out=g2[:], in_=g1[:], func=ACTF.Exp, scale=-1.0)
    g5 = sp.tile([128, GCOLS], F32, tag="g5")
    nc.vector.scalar_tensor_tensor(
        out=g5[:], in0=go, scalar=0.0, in1=gt,
        op0=ALU.add, op1=ALU.mult, accum_out=spst[:, 0:1])
    g6 = sp.tile([128, GCOLS], F32, tag="g6")
    nc.vector.scalar_tensor_tensor(
        out=g6[:], in0=go, scalar=0.0, in1=go,
        op0=ALU.is_gt, op1=ALU.mult, accum_out=spst[:, 1:2])
    bscr = sp.tile([128, nb], BF16, tag="bscr")
    nc.vector.scalar_tensor_tensor(
        out=bscr[:], in0=band_sb, scalar=1.0, in1=bw_sb,
        op0=ALU.mult, op1=ALU.mult, accum_out=spst[:, 2:3])

    ones_v = ones2[:].rearrange("p (two s) -> p two s", two=2)[:, :, 0:1]
    id2 = id_sb.rearrange("p (two m) -> p two m", two=2)

    # --- mel L1 (PE diffs -> ACT/DVE abs+row-sum).  All mel DMAs go first:
    # the psum readers are the long pole, so their stream starts ASAP.
    # The canvas only feeds the (cheap) PE colsum chain and rides last. ---
    ncols = {'A': 0, 'D': 0}
    for kh in range(NMT * 2):
        mt = melp.tile([128, MELW], F8, tag="mel")
        nc.sync.dma_start(out=mt[:], in_=d_mel[:, kh * MELW:(kh + 1) * MELW])
        for p in range(2):
            # pair 0: planes (mo, mt) -> mo - mt; pair 1: (mt, mp) -> mt - mp
            pv = mt[:, p * MELS:p * MELS + 2 * MELS].rearrange(
                "p (two j) -> p two j", two=2)
            ps = psmel.tile([128, 1024], F32, tag="mps")
            nc.tensor.matmul(ps[:, 0:512], id2, pv[:, :, 0:512],
                             start=True, stop=True, perf_mode=DR,
                             skip_group_check=True)
            nc.tensor.matmul(ps[:, 512:MELH], id2, pv[:, :, 512:MELH],
                             start=True, stop=True, perf_mode=DR,
                             skip_group_check=True)
            eng = READER[kh * 2 + p]
            col = ncols[eng]
            ncols[eng] += 1
            if eng == 'A':
                scr = scrp.tile([128, MELH], BF16, tag="scr")
                nc.scalar.activation(out=scr[:], in_=ps[:, 0:MELH],
                                     func=ACTF.Abs,
                                     accum_out=sa[:, col:col + 1])
            else:
                nc.vector.tensor_reduce(
                    out=sd[:, col:col + 1], in_=ps[:, 0:MELH],
                    axis=mybir.AxisListType.X, op=ALU.add,
                    apply_absolute_value=True)

    # --- canvas column sums (ones-stationary DoubleRow chain) ---
    total_cv = d_chunks * 512
    n_groups = (d_chunks + 1) // 2
    ones1 = ones2[:, 0:1]
    cv_pair = 0
    off = 0
    while off < total_cv:
        w = min(2048, total_cv - off)
        cvt = cvp.tile([128, 2048], F8, tag="cv")
        nc.sync.dma_start(out=cvt[:, 0:w], in_=d_cv[:, off:off + w])
        h = 0
        while h * 1024 < w:
            first = cv_pair == 0
            last = cv_pair == n_groups - 1
            if w - h * 1024 >= 1024:
                nc.tensor.matmul(
                    cs_ps[:], ones_v,
                    cvt[:, h * 1024:(h + 1) * 1024].rearrange(
                        "p (two j) -> p two j", two=2),
                    start=first, stop=last,
                    perf_mode=DR, skip_group_check=True)
            else:
                # odd trailing 512-col chunk: plain fp8 matmul
                nc.tensor.matmul(
                    cs_ps[:], ones1, cvt[:, h * 1024:h * 1024 + 512],
                    start=first, stop=last, skip_group_check=True)
            cv_pair += 1
            h += 1
        off += w

    # --- gate BCE tail: ln(1 + exp(-|x|)) accum (needs the natural_log
    # activation table; the load lands after the mel Abs stream ends) ---
    g3 = sp.tile([128, GCOLS], F32, tag="g3")
    nc.scalar.activation(out=g3[:], in_=g2[:], func=ACTF.Ln, bias=1.0,
                         accum_out=sa[:, 10:11])


_PROGRAMS = {}


def _get_program(d_chunks=None, nb=None, n_reps=1):
    if d_chunks is None or nb is None:
        assert _LAYOUT is not None, "call kernel() first"
        d_chunks, _, nb = _LAYOUT
    key = (d_chunks, nb, n_reps)
    if key not in _PROGRAMS:
        _PROGRAMS[key] = _build_program(d_chunks, nb, n_reps)
    return _PROGRAMS[key]


def _build_program_reps(n_reps):
    assert _LAYOUT is not None, "call kernel() (or _prep_core) first"
    d, _, nb = _LAYOUT
    return _get_program(d, nb, n_reps)


def _core_box_count(in_len, out_len):
    return int(np.sum(out_len.astype(np.int64) * (TIN - in_len.astype(np.int64))))


def _core_band_cols(out_len):
    return -(-int(np.sum(out_len.astype(np.int64))) * BW // 128)


def _prep_core(al, melo, melp_, melt, go, gt, in_len, out_len):
    """Build one core's input map. al: [BPC, TOUT, TIN] etc. (numpy f32)."""
    global _LAYOUT
    in_len = np.asarray(in_len, dtype=np.int64)
    out_len = np.asarray(out_len, dtype=np.int64)
    if _LAYOUT is None:
        # standalone use: size from this core with margin
        _LAYOUT = _canvas_layout(int(_core_box_count(in_len, out_len) * 1.25),
                                 _core_band_cols(out_len) + 64)
    d, ranges, nb = _LAYOUT

    # mel: per (row-tile k, half h): [mo | mt | mp] planes of MELH cols
    # padded to MELS so the DoubleRow plane stride is a multiple of 16
    m3 = np.stack([melo.reshape(MROWS, TOUT),
                   melt.reshape(MROWS, TOUT),
                   melp_.reshape(MROWS, TOUT)], axis=1)     # [640, 3, 2000]
    m4 = np.zeros((NMT, 128, 2, 3, MELS), np.float32)
    m5 = m3.reshape(NMT, 128, 3, 2, MELH)                   # [k, p, t, h, j]
    m4[:, :, :, :, 0:MELH] = m5.transpose(0, 1, 3, 2, 4)
    mel8 = np.ascontiguousarray(
        m4.transpose(1, 0, 2, 3, 4).reshape(128, NMT * 2 * MELW)).astype(F8NP)

    # canvas groups
    box_vals = [al[s, :out_len[s], in_len[s]:] for s in range(BPC)]
    box = (np.concatenate([v.ravel() for v in box_vals])
           if box_vals else np.zeros(0, np.float32))
    dirv = np.concatenate([al[s, :IMID + 1, :IMID][_DIR_MASK]
                           for s in range(BPC)])
    tailv = np.concatenate([al[s, IMID + 1:TIN, :][_TAIL_MASK]
                            for s in range(BPC)])

    cv = np.zeros((d, 512, 128), np.float32)
    for vals, (a, b) in zip((box, dirv, tailv), ranges):
        cap = d * (b - a) * 128
        assert len(vals) <= cap, f"canvas overflow: {len(vals)} > {cap}"
        pad = np.zeros(cap, np.float32)
        pad[:len(vals)] = vals * ASCALE
        cv[:, a:b, :] = pad.reshape(d, b - a, 128)
    cv8 = np.ascontiguousarray(cv.transpose(2, 0, 1).reshape(128, d * 512)
                               ).astype(F8NP)

    # gaussian band: 4 columns around j* = i*in/out for valid rows
    bands = []
    bws = []
    for s in range(BPC):
        ol, il = int(out_len[s]), int(in_len[s])
        iv = np.arange(ol, dtype=np.float64)
        jstar = iv * il / ol
        s0 = np.clip(np.floor(jstar).astype(np.int64) - 1, 0, TIN - BW)
        jb = s0[:, None] + np.arange(BW)[None, :]            # [ol, BW]
        bands.append(al[s, iv.astype(np.int64)[:, None], jb].ravel())
        dlt = iv[:, None] - jb * (float(ol) / il)
        w = np.exp(ESCALE * dlt * dlt)
        w[jb >= il] = 0.0
        bws.append(w.ravel())
    bflat = np.concatenate(bands)
    wflat = np.concatenate(bws)
    bpad = np.zeros(128 * nb, np.float32)
    bpad[:len(bflat)] = bflat
    wpad = np.zeros(128 * nb, np.float32)
    wpad[:len(wflat)] = wflat

    # identity stationary: [p, 0*128+m]=+1[p==m], [p, 128+m]=-1[p==m]
    idw = np.zeros((128, 256), np.float32)
    idw[np.arange(128), np.arange(128)] = 1.0
    idw[np.arange(128), 128 + np.arange(128)] = -1.0

    gate = np.ascontiguousarray(
        np.concatenate([go.reshape(128, GCOLS), gt.reshape(128, GCOLS)],
                       axis=1), dtype=np.float32)
    u8 = np.uint8
    aux = np.concatenate([
        np.ascontiguousarray(idw.astype(F8NP)).view(u8),
        gate.view(u8),
        np.ascontiguousarray(bpad.reshape(128, nb).astype(BFNP)).view(u8),
        np.ascontiguousarray(wpad.reshape(128, nb).astype(BFNP)).view(u8),
    ], axis=1)

    return {"aux": np.ascontiguousarray(aux), "mel": mel8, "cv": cv8}


def kernel(mel_out, mel_out_postnet, gate_out, alignments,
           mel_target, gate_target, input_lengths, output_lengths,
           _results_hook=None):
    global _LAYOUT
    mel_out = np.asarray(mel_out, dtype=np.float32)
    mel_out_postnet = np.asarray(mel_out_postnet, dtype=np.float32)
    gate_out = np.asarray(gate_out, dtype=np.float32)
    alignments = np.asarray(alignments, dtype=np.float32)
    mel_target = np.asarray(mel_target, dtype=np.float32)
    gate_target = np.asarray(gate_target, dtype=np.float32)
    in_len = np.asarray(input_lengths).astype(np.int64)
    out_len = np.asarray(output_lengths).astype(np.int64)

    # global layout from all cores (one SPMD program)
    max_box = 0
    max_nb = 0
    for c in range(NCORES):
        sl = slice(BPC * c, BPC * (c + 1))
        max_box = max(max_box, _core_box_count(in_len[sl], out_len[sl]))
        max_nb = max(max_nb, _core_band_cols(out_len[sl]))
    lay = _canvas_layout(max_box, max_nb)
    if _LAYOUT is None or _LAYOUT[0] < lay[0] or _LAYOUT[2] < lay[2]:
        _LAYOUT = lay
    d, ranges, nb = _LAYOUT

    in_maps = []
    for c in range(NCORES):
        sl = slice(BPC * c, BPC * (c + 1))
        in_maps.append(_prep_core(
            alignments[sl], mel_out[sl], mel_out_postnet[sl], mel_target[sl],
            gate_out[sl], gate_target[sl], in_len[sl], out_len[sl]))

    nc = _get_program(d, nb)
    res = run_bass_kernel_spmd(nc, in_maps, core_ids=list(range(NCORES)))
    if _results_hook is not None:
        _results_hook(res)

    mel_sum = gsp = grelu = gxz = gauss = 0.0
    att = box = 0.0
    (ba, bb), (da, db), (ta, tb) = ranges
    for c in range(NCORES):
        out = res.results[c]
        sa = out["sa"].astype(np.float64)
        sd = out["sd"].astype(np.float64)
        spst = out["sp"].astype(np.float64)
        cs = out["cs"].astype(np.float64)[0]

        mel_sum += sa[:, 0:10].sum() + sd[:, 0:10].sum()
        gsp += sa[:, 10].sum()
        gxz += spst[:, 0].sum()
        grelu += spst[:, 1].sum()
        gauss += spst[:, 2].sum()

        box_tail = cs[ba:bb].sum() / ASCALE
        att_dir = cs[da:db].sum() / ASCALE
        att_tail = cs[ta:tb].sum() / ASCALE

        sl = slice(BPC * c, BPC * (c + 1))
        att += BPC * ATT_CONST + att_dir - att_tail
        box += float(out_len[sl].sum()) - box_tail

    n_mel = B * MEL * TOUT
    n_gate = B * TOUT
    mel_loss = mel_sum / n_mel
    gate_loss = (grelu - gxz + gsp) / n_gate
    att_loss = att / B
    ga_loss = (box - gauss) / B
    total = (MEL_W * mel_loss + GATE_W * gate_loss
             + ATT_W * att_loss + GA_W * ga_loss)
    f = np.float32
    return (f(total), f(mel_loss), f(gate_loss), f(att_loss), f(ga_loss))
